# revision 1
# baseline (speedup 1.0000x reference)
"""Edge-parallel GNN message-passing kernel for 8 Trainium2 NeuronCores.

Strategy:
  * Host: sort edges by destination, split nodes into 8 contiguous ranges;
    each core gets every edge whose destination is in its range.
  * The node-level src/dst linear transforms are folded into the first edge
    MLP layer on the host (they are linear, no activation in between):
      h1_pre = fsrc @ (W_src@W1a) + fdst @ (W_dst@W1b) + b1f
      g1_pre = fsrc @ (W_src@Wg1a) + fdst @ (W_dst@Wg1b) + bg1f
  * Device, per 512-edge group: gather fp16 feature rows transposed
    ([feat, edge] layout) with dma_gather, run the MLP/gate/LayerNorm with
    fp32 PSUM accumulation, segment-sum the gated messages into a 256-node
    window via a one-hot matmul, apply W_out, write a dense staging tile.
  * Host: accumulate the (overlapping) staging windows into the full
    [N, DOUT] output and add b_out.
"""

import sys

sys.path.insert(0, "/opt/trn_rl_repo")

import numpy as np

import concourse.bass as bass
import concourse.bacc as bacc
import concourse.tile as tile
from concourse import mybir
from concourse.bass_utils import run_bass_kernel_spmd

N_CORES = 8
H = 128
WINDOW = 256          # destination-node window per group (PSUM free dim)
GROUP_E = 512         # edges per group
CHUNK_G = 4           # groups per dma_gather call (2048 indices)
CHUNK_E = GROUP_E * CHUNK_G
LN_EPS = 1e-5
F16 = mybir.dt.float16
F32 = mybir.dt.float32
I16 = mybir.dt.int16
I32 = mybir.dt.int32
AF = mybir.ActivationFunctionType
ALU = mybir.AluOpType


# --------------------------------------------------------------------------
# host-side packing
# --------------------------------------------------------------------------

def _pack_core(ed_c, es_c, core_base):
    """Split one core's dst-sorted edges into groups of <=GROUP_E edges whose
    destinations span <WINDOW nodes.  Returns per-group (start, end, base)."""
    out = []
    i = 0
    n = len(ed_c)
    while i < n:
        base = int(ed_c[i])
        j_window = int(np.searchsorted(ed_c, base + WINDOW, side="left"))
        j = min(i + GROUP_E, j_window)
        out.append((i, j, base))
        i = j
    return out


def _prepare(inputs):
    feat = np.ascontiguousarray(np.asarray(inputs["feat"], np.float32))
    es = np.asarray(inputs["edge_src"]).astype(np.int64)
    ed = np.asarray(inputs["edge_dst"]).astype(np.int64)
    N, DIN = feat.shape
    E = es.shape[0]
    npc = -(-N // N_CORES)  # nodes per core (ceil)

    f64 = np.float64
    W_src = np.asarray(inputs["W_src"], f64)
    W_dst = np.asarray(inputs["W_dst"], f64)
    W1a = np.asarray(inputs["W1a"], f64)
    W1b = np.asarray(inputs["W1b"], f64)
    Wg1a = np.asarray(inputs["Wg1a"], f64)
    Wg1b = np.asarray(inputs["Wg1b"], f64)
    b_src = np.asarray(inputs["b_src"], f64)
    b_dst = np.asarray(inputs["b_dst"], f64)
    ln_g = np.asarray(inputs["ln_g"], f64)
    ln_b = np.asarray(inputs["ln_b"], f64)
    if not np.allclose(ln_b, 0.0):
        raise NotImplementedError("non-zero ln_b not supported")

    wpack = {
        "A1s": W_src @ W1a,
        "A1d": W_dst @ W1b,
        "Ag1s": W_src @ Wg1a,
        "Ag1d": W_dst @ Wg1b,
        "W2": np.asarray(inputs["W2"], f64),
        "W3": np.asarray(inputs["W3"], f64),
        "W_out": np.diag(ln_g) @ np.asarray(inputs["W_out"], f64),
    }
    b1f = np.asarray(inputs["b1"], f64) + b_src @ W1a + b_dst @ W1b
    bg1f = np.asarray(inputs["bg1"], f64) + b_src @ Wg1a + b_dst @ Wg1b
    Wg2 = np.asarray(inputs["Wg2"], f64)  # [H, 1]
    bg2 = float(np.asarray(inputs["bg2"], f64).reshape(()))

    feat16 = feat.astype(np.float16)

    order = np.argsort(ed, kind="stable")
    es_s = es[order]
    ed_s = ed[order]
    bounds = np.searchsorted(ed_s, np.arange(N_CORES + 1) * npc, side="left")

    core_groups = []
    for c in range(N_CORES):
        lo, hi = int(bounds[c]), int(bounds[c + 1])
        core_groups.append(_pack_core(ed_s[lo:hi], es_s[lo:hi], c * npc))

    G = max(len(g) for g in core_groups)

    # pick NTAB so every per-(core, table) unique-source count fits in int16
    for ntab in (2, 4, 8, 16):
        Gp = -(-G // (CHUNK_G * ntab)) * (CHUNK_G * ntab)
        nchunk = Gp // CHUNK_G
        ok = True
        max_u = 0
        per_core = []
        for c in range(N_CORES):
            lo = int(bounds[c])
            groups = core_groups[c]
            # per-group source ids (global), pads use the group's first src
            srcids = np.zeros((Gp, GROUP_E), np.int64)
            lidx = np.full((Gp, GROUP_E), -1.0, np.float32)
            bases = np.zeros(Gp, np.int64)
            for g, (i, j, base) in enumerate(groups):
                k = j - i
                seg_src = es_s[lo + i: lo + j]
                srcids[g, :k] = seg_src
                srcids[g, k:] = seg_src[0] if k else 0
                lidx[g, :k] = (ed_s[lo + i: lo + j] - base).astype(np.float32)
                bases[g] = base
            tabs = []
            sidx = np.zeros((Gp, GROUP_E), np.int64)
            cpt = nchunk // ntab  # chunks per table
            for t in range(ntab):
                g0, g1 = t * cpt * CHUNK_G, (t + 1) * cpt * CHUNK_G
                ids = srcids[g0:g1].ravel()
                uniq, inv = np.unique(ids, return_inverse=True)
                max_u = max(max_u, len(uniq))
                if len(uniq) > 32767:
                    ok = False
                    break
                tabs.append(uniq)
                sidx[g0:g1] = inv.reshape(g1 - g0, GROUP_E)
            if not ok:
                break
            per_core.append((srcids, sidx, lidx, bases, tabs))
        if ok:
            break
    if not ok:
        raise RuntimeError("could not fit source tables into int16 indices")

    U = max(len(t) for (_, _, _, _, tabs) in per_core for t in tabs)
    U = -(-U // 128) * 128

    # device input arrays per core
    in_maps = []
    meta = []
    for c in range(N_CORES):
        srcids, sidx, lidx, bases, tabs = per_core[c]
        lo = int(bounds[c])
        srctabs = np.zeros((ntab, U, H), np.float16)
        for t, uniq in enumerate(tabs):
            srctabs[t, : len(uniq)] = feat16[uniq]
        c0 = c * npc
        dsttab = np.zeros((npc, H), np.float16)
        seg = feat16[c0: min(c0 + npc, N)]
        dsttab[: len(seg)] = seg

        didx = np.zeros((Gp, GROUP_E), np.int64)
        for g, (i, j, base) in enumerate(core_groups[c]):
            k = j - i
            didx[g, :k] = ed_s[lo + i: lo + j] - c0

        def wrap16(idx):  # [Gp, GROUP_E] -> [nchunk, 128, CHUNK_E//16]
            a = idx.reshape(nchunk, CHUNK_E // 16, 16).transpose(0, 2, 1)
            return np.tile(a, (1, 8, 1)).astype(np.int16)

        # lidx transposed for per-partition access: [128, 4*Gp]
        lidxT = np.ascontiguousarray(
            lidx.reshape(Gp, CHUNK_G, 128).transpose(2, 0, 1).reshape(128, -1)
        ).astype(np.float32)

        im = {
            "srctabs": srctabs,
            "dsttab": dsttab,
            "sidxw": wrap16(sidx),
            "didxw": wrap16(didx),
            "lidxT": lidxT,
            "b3rep": np.tile(np.asarray(inputs["b3"], np.float32), (128, CHUNK_G)),
            "iota": np.tile(np.arange(WINDOW, dtype=np.float16), (128, 1)),
            "b1f": b1f.astype(np.float32).reshape(H, 1),
            "bg1f": bg1f.astype(np.float32).reshape(H, 1),
            "b2": np.asarray(inputs["b2"], np.float32).reshape(H, 1),
            "bg2": np.full((128, 1), 0.5 * bg2, np.float32),  # tanh-form sigmoid
            "Wg2": Wg2.astype(np.float16),
        }
        for k, v in wpack.items():
            im[k] = v.astype(np.float16)
        in_maps.append(im)
        meta.append(bases)

    b_out = np.asarray(inputs["b_out"], np.float64)
    return dict(
        in_maps=in_maps, meta=meta, Gp=Gp, nchunk=nchunk, ntab=ntab, U=U,
        npc=npc, N=N, b_out=b_out,
    )


# --------------------------------------------------------------------------
# device kernel builder
# --------------------------------------------------------------------------

def _build(Gp, nchunk, ntab, U, npc, reps=1):
    nc = bacc.Bacc("TRN2", target_bir_lowering=False, debug=False)
    d = {}
    d["srctabs"] = nc.dram_tensor("srctabs", [ntab, U, H], F16, kind="ExternalInput")
    d["dsttab"] = nc.dram_tensor("dsttab", [npc, H], F16, kind="ExternalInput")
    d["sidxw"] = nc.dram_tensor("sidxw", [nchunk, 128, CHUNK_E // 16], I16,
                                kind="ExternalInput")
    d["didxw"] = nc.dram_tensor("didxw", [nchunk, 128, CHUNK_E // 16], I16,
                                kind="ExternalInput")
    d["lidxT"] = nc.dram_tensor("lidxT", [128, CHUNK_G * Gp], F32,
                                kind="ExternalInput")
    d["b3rep"] = nc.dram_tensor("b3rep", [128, CHUNK_G * 128], F32,
                                kind="ExternalInput")
    d["iota"] = nc.dram_tensor("iota", [128, WINDOW], F16, kind="ExternalInput")
    for nm in ("b1f", "bg1f", "b2", "bg2"):
        d[nm] = nc.dram_tensor(nm, [128, 1], F32, kind="ExternalInput")
    for nm in ("A1s", "A1d", "Ag1s", "Ag1d", "W2", "W3", "W_out"):
        d[nm] = nc.dram_tensor(nm, [H, H], F16, kind="ExternalInput")
    d["Wg2"] = nc.dram_tensor("Wg2", [H, 1], F16, kind="ExternalInput")
    staging = nc.dram_tensor("staging", [Gp, 2, 128, 128], F16,
                             kind="ExternalOutput")

    with tile.TileContext(nc) as tc:
        with (
            tc.tile_pool(name="singles", bufs=1) as singles,
            tc.tile_pool(name="gath", bufs=2) as gath,
            tc.tile_pool(name="acts", bufs=3) as acts,
            tc.tile_pool(name="ln", bufs=3) as lnp,
            tc.tile_pool(name="outp", bufs=3) as outp,
            tc.tile_pool(name="ppack", bufs=3, space="PSUM") as ppack,
            tc.tile_pool(name="psmall", bufs=2, space="PSUM") as psmall,
        ):
            # ---- preamble: constants into SBUF ----
            w = {}
            for nm in ("A1s", "A1d", "Ag1s", "Ag1d", "W2", "W3", "W_out"):
                w[nm] = singles.tile([H, H], F16, tag=nm, name=nm)
                nc.sync.dma_start(out=w[nm], in_=d[nm][:, :])
            w["Wg2"] = singles.tile([H, 1], F16, tag="Wg2", name="Wg2")
            nc.sync.dma_start(out=w["Wg2"], in_=d["Wg2"][:, :])
            bias = {}
            for nm in ("b1f", "bg1f", "b2", "bg2"):
                bias[nm] = singles.tile([128, 1], F32, tag=nm, name=nm)
                nc.sync.dma_start(out=bias[nm], in_=d[nm][:, :])
            b3rep = singles.tile([128, CHUNK_G * 128], F32, tag="b3rep")
            nc.sync.dma_start(out=b3rep, in_=d["b3rep"][:, :])
            iota = singles.tile([128, WINDOW], F16, tag="iota")
            nc.sync.dma_start(out=iota, in_=d["iota"][:, :])
            lidxT = singles.tile([128, CHUNK_G * Gp], F32, tag="lidxT")
            nc.sync.dma_start(out=lidxT, in_=d["lidxT"][:, :])
            sidx_sb = singles.tile([128, nchunk, CHUNK_E // 16], I16, tag="sidx")
            nc.sync.dma_start(
                out=sidx_sb,
                in_=d["sidxw"].rearrange("c p e -> p c e"),
            )
            didx_sb = singles.tile([128, nchunk, CHUNK_E // 16], I16, tag="didx")
            nc.sync.dma_start(
                out=didx_sb,
                in_=d["didxw"].rearrange("c p e -> p c e"),
            )
            # integer constants for the Quake-style rsqrt seed (GPSIMD)
            c_one = singles.tile([128, CHUNK_G * CHUNK_G], I32, tag="c_one")
            nc.vector.memset(c_one, 1)
            c_neg1 = singles.tile([128, CHUNK_G * CHUNK_G], I32, tag="c_neg1")
            nc.vector.memset(c_neg1, -1)
            c_magic = singles.tile([128, CHUNK_G * CHUNK_G], I32, tag="c_magic")
            nc.vector.memset(c_magic, 0x5F3759E0)

            cpt = nchunk // ntab
            for _rep in range(reps):
              for c in range(nchunk):
                fsT = gath.tile([128, 1, CHUNK_E], F16, tag="fsT")
                nc.gpsimd.dma_gather(
                    out_ap=fsT,
                    in_ap=d["srctabs"][c // cpt],
                    idxs_ap=sidx_sb[:, c, :],
                    num_idxs=CHUNK_E,
                    num_idxs_reg=CHUNK_E,
                    elem_size=H,
                    transpose=True,
                    single_packet=False,
                    queue_num=0,
                )
                fdT = gath.tile([128, 1, CHUNK_E], F16, tag="fdT")
                nc.gpsimd.dma_gather(
                    out_ap=fdT,
                    in_ap=d["dsttab"][:, :],
                    idxs_ap=didx_sb[:, c, :],
                    num_idxs=CHUNK_E,
                    num_idxs_reg=CHUNK_E,
                    elem_size=H,
                    transpose=True,
                    single_packet=False,
                    queue_num=0,
                )
                # -- phase A: edge MLP + LN stats, per group; gate matmuls
                #    accumulate into one chunk-wide PSUM tile --
                gatep = psmall.tile([128, CHUNK_G * CHUNK_G], F32, tag="gatep", bufs=2)
                mv = lnp.tile([128, CHUNK_G * CHUNK_G, 2], F32, tag="mv")
                xs = []
                for gi in range(CHUNK_G):
                    e0 = gi * GROUP_E
                    fs = fsT[:, 0, e0:e0 + GROUP_E]
                    fd = fdT[:, 0, e0:e0 + GROUP_E]

                    h1p = ppack.tile([128, GROUP_E], F32, tag="big")
                    nc.tensor.matmul(h1p, w["A1s"], fs, start=True, stop=False)
                    nc.tensor.matmul(h1p, w["A1d"], fd, start=False, stop=True)
                    g1p = ppack.tile([128, GROUP_E], F32, tag="big")
                    nc.tensor.matmul(g1p, w["Ag1s"], fs, start=True, stop=False)
                    nc.tensor.matmul(g1p, w["Ag1d"], fd, start=False, stop=True)

                    h1s = acts.tile([128, GROUP_E], F16, tag="h1s")
                    nc.scalar.activation(h1s, h1p, AF.Gelu, bias=bias["b1f"])
                    h2p = ppack.tile([128, GROUP_E], F32, tag="big")
                    nc.tensor.matmul(h2p, w["W2"], h1s, start=True, stop=True)
                    h2s = acts.tile([128, GROUP_E], F16, tag="h2s")
                    nc.scalar.activation(h2s, h2p, AF.Gelu, bias=bias["b2"])
                    g1s = acts.tile([128, GROUP_E], F16, tag="g1s")
                    nc.scalar.activation(g1s, g1p, AF.Gelu, bias=bias["bg1f"])

                    # msg_pre (un-transposed, [edge, feat]) and gate pre-act
                    msgp = ppack.tile([128, GROUP_E], F32, tag="big")
                    for s in range(4):
                        sl = slice(s * 128, (s + 1) * 128)
                        nc.tensor.matmul(
                            msgp[:, sl], h2s[:, sl], w["W3"],
                            start=True, stop=True, skip_group_check=True,
                        )
                        k = gi * CHUNK_G + s
                        nc.tensor.matmul(
                            gatep[:, k:k + 1], g1s[:, sl], w["Wg2"],
                            start=True, stop=True, skip_group_check=True,
                        )
                    # x = msg_pre + b3 (fp16), then per-subtile LN stats
                    x = lnp.tile([128, CHUNK_G, 128], F16, tag="x", bufs=8)
                    nc.vector.tensor_tensor(
                        x, msgp.rearrange("p (s f) -> p s f", s=CHUNK_G),
                        b3rep.rearrange("p (s f) -> p s f", s=CHUNK_G),
                        op=ALU.add,
                    )
                    xs.append(x)
                    st = lnp.tile([128, CHUNK_G, 6], F32, tag="st")
                    for s in range(4):
                        k = gi * CHUNK_G + s
                        nc.vector.bn_stats(st[:, s, :], x[:, s, :])
                        nc.vector.bn_aggr(mv[:, k, :], st[:, s, :])

                # -- phase B (chunk-wide, on GPSIMD): gate = 0.5*(tanh+1),
                #    rstd via Quake seed + 2 Newton iterations, sc = gate*rstd
                #    (the 0.5s are folded into the final Newton step) --
                NG = CHUNK_G * CHUNK_G
                gate_t = lnp.tile([128, NG], F32, tag="gate_t")
                nc.scalar.activation(gate_t, gatep, AF.Tanh,
                                     bias=bias["bg2"], scale=0.5)
                g2t = lnp.tile([128, NG], F32, tag="g2t")
                nc.gpsimd.tensor_scalar(g2t, gate_t, 1.0, None, op0=ALU.add)
                wt = lnp.tile([128, NG], F32, tag="wt")
                nc.gpsimd.tensor_scalar(wt, mv[:, :, 1], LN_EPS, None,
                                        op0=ALU.add)
                ya = lnp.tile([128, NG], F32, tag="ya")
                yb = lnp.tile([128, NG], F32, tag="yb")
                tmp = lnp.tile([128, NG], F32, tag="tmp")
                ya_i, yb_i = ya.bitcast(I32), yb.bitcast(I32)
                nc.vector.tensor_tensor(ya_i, wt.bitcast(I32), c_one,
                                        op=ALU.logical_shift_right)
                nc.vector.tensor_tensor(yb_i, ya_i, c_neg1, op=ALU.bitwise_xor)
                nc.vector.tensor_tensor(ya_i, yb_i, c_magic, op=ALU.add)
                # Newton iter 1: ya = ya*(1.5 - 0.5*wt*ya^2)
                nc.gpsimd.tensor_tensor(tmp, ya, ya, op=ALU.mult)
                nc.gpsimd.tensor_tensor(tmp, tmp, wt, op=ALU.mult)
                nc.gpsimd.tensor_scalar(tmp, tmp, -0.5, 1.5,
                                        op0=ALU.mult, op1=ALU.add)
                nc.gpsimd.tensor_tensor(yb, ya, tmp, op=ALU.mult)
                # Newton iter 2 with 0.5 folded: ya = yb*(0.75 - 0.25*wt*yb^2)
                nc.gpsimd.tensor_tensor(tmp, yb, yb, op=ALU.mult)
                nc.gpsimd.tensor_tensor(tmp, tmp, wt, op=ALU.mult)
                nc.gpsimd.tensor_scalar(tmp, tmp, -0.25, 0.75,
                                        op0=ALU.mult, op1=ALU.add)
                nc.gpsimd.tensor_tensor(ya, yb, tmp, op=ALU.mult)
                sc = lnp.tile([128, NG], F32, tag="sc")
                nc.gpsimd.tensor_tensor(sc, ya, g2t, op=ALU.mult)

                # -- phase C: center, one-hot (scaled), segment-sum, W_out --
                for gi in range(CHUNK_G):
                    g = c * CHUNK_G + gi
                    x = xs[gi]
                    msg16 = acts.tile([128, GROUP_E], F16, tag="msg16")
                    A = acts.tile([128, CHUNK_G, WINDOW], F16, tag="A")
                    for s in range(4):
                        sl = slice(s * 128, (s + 1) * 128)
                        k = gi * CHUNK_G + s
                        nc.vector.tensor_scalar(
                            msg16[:, sl], x[:, s, :],
                            mv[:, k, 0:1], None, op0=ALU.subtract,
                        )
                        nc.vector.tensor_scalar(
                            A[:, s, :], iota,
                            lidxT[:, g * CHUNK_G + s: g * CHUNK_G + s + 1],
                            sc[:, k:k + 1],
                            op0=ALU.is_equal, op1=ALU.mult,
                        )

                    updp = psmall.tile([128, WINDOW], F32, tag="sm")
                    for s in range(4):
                        sl = slice(s * 128, (s + 1) * 128)
                        nc.tensor.matmul(
                            updp, msg16[:, sl], A[:, s, :],
                            start=(s == 0), stop=(s == 3),
                            skip_group_check=True,
                        )
                    upd16 = outp.tile([128, WINDOW], F16, tag="upd16")
                    if gi % 2 == 0:
                        nc.vector.tensor_copy(upd16, updp)
                    else:
                        nc.scalar.activation(upd16, updp, AF.Copy)

                    o2 = psmall.tile([128, 2, 128], F32, tag="o2", bufs=1)
                    for hh in range(2):
                        nc.tensor.matmul(
                            o2[:, hh, :], upd16[:, hh * 128:(hh + 1) * 128],
                            w["W_out"], start=True, stop=True,
                            skip_group_check=True,
                        )
                    osb = outp.tile([128, 2, 128], F16, tag="osb")
                    if gi % 2 == 0:
                        nc.scalar.activation(osb, o2, AF.Copy)
                    else:
                        nc.vector.tensor_copy(osb, o2)
                    nc.sync.dma_start(
                        out=staging[g].rearrange("hh j d -> j hh d"),
                        in_=osb,
                    )
    nc.finalize()
    return nc


# --------------------------------------------------------------------------
# entry point
# --------------------------------------------------------------------------

_LAST_PERF = {}


def kernel(**inputs):
    prep = _prepare(inputs)
    nc = _build(prep["Gp"], prep["nchunk"], prep["ntab"], prep["U"], prep["npc"])
    import os
    import time as _time
    trace = bool(int(os.environ.get("KERNEL_TRACE", "0")))
    res = run_bass_kernel_spmd(
        nc, prep["in_maps"], core_ids=list(range(N_CORES)), trace=trace,
    )
    if int(os.environ.get("KERNEL_REPEAT", "0")):
        t0 = _time.time()
        res = run_bass_kernel_spmd(
            nc, prep["in_maps"], core_ids=list(range(N_CORES)), trace=trace,
        )
        _rw = _time.time() - t0
    else:
        _rw = None
    _LAST_PERF.clear()
    _LAST_PERF.update(
        repeat_wall_s=_rw,
        exec_time_ns=res.exec_time_ns,
        mean_exec_time_ns=res.mean_exec_time_ns,
        trace=res.instructions_and_trace[1] if res.instructions_and_trace else None,
    )

    N = prep["N"]
    out = np.zeros((N + WINDOW, H), np.float64)
    for c in range(N_CORES):
        stg = res.results[c]["staging"].reshape(prep["Gp"], WINDOW, H)
        bases = prep["meta"][c]
        for g in range(prep["Gp"]):
            b = int(bases[g])
            out[b: b + WINDOW] += stg[g]
    out = out[:N] + prep["b_out"]
    return out.astype(np.float32)

